# revision 3
# baseline (speedup 1.0000x reference)
"""Trainium2 Bass kernel for additive attention (nn_AdditiveAttention).

Reference computation (per batch b):
    q_proj = query @ W1_w.T + W1_b                      # [D]
    v_proj = values @ W2_w.T + W2_b                     # [T, D]
    scores = tanh(q_proj + v_proj) @ v                  # [T]
    weights = softmax(scores)                           # [T]
    out    = weights @ values                           # [E]

Sharding: data-parallel over batch B=32 across 8 NeuronCores (4 batches/core).

V7 design (evolves V6). `values` is consumed ONLY in transposed [e, t]
layout (host-pretransposed contiguous 32MB bf16 stream per core). Per group
of 2048 timesteps:

  - DMA vt [128e, 4, 2048t] bf16 (one 2MB load)
  - v_proj: 4 psum tiles [128d(chunk dc), 512t], dc-outer / e-chunk-inner so
    each W2 chunk is loaded once per pass and reused for 4 super matmuls
  - ACT tanh w/ per-partition bias -> th bf16
  - scores: lhsT is v REPLICATED into a [128, 128] stationary operand, so
    the score matmul writes the score row to ALL 128 psum partitions for the
    same column count (cost unchanged). 8 MMs per group -> 4 full
    [128, 512] psum banks.
  - softmax WITHOUT max subtraction (|scores| <~ 25 so exp fits f32):
    ACT exp on the replicated psum -> weight tile wb [128, 4, 512] bf16
    DIRECTLY in broadcast layout (the V6 gpsimd partition_broadcast and the
    exp accum_out are both gone). Row 0 of wb is DMA'd out per group; the
    host sums it for the denominator.
  - numerator: 4 fused mult+reduce [128, 2048] ops,
      num_col[p, c] = sum_t vt[p, c, t] * wb[p, t]   (e = c*128+p)
    split 3 on DVE + 1 on GPSIMD (engine budget: DVE ~105us, Pool ~47us).
  - per (batch, group) outputs: osb [128, EC] f32 num cols + wb row 0.

Engine budget per core: PE ~138us (v_proj 109 + scores 28) <- ceiling,
DVE ~105us, ACT ~101us (tanh 65 + exp 36), DMA ~95us, Pool ~47us.

All matmul operands are bf16 (~4e-3 rel err; harness gate is 2e-2).
"""

import os
import sys
import time

import numpy as np

for _p in ("/opt/trn_rl_repo",):
    if _p not in sys.path and os.path.isdir(_p):
        sys.path.insert(0, _p)

# Problem shapes (hardcoded per contract)
B, T, E, D = 32, 8192, 512, 256
N_CORES = 8
B_LOC = B // N_CORES          # 4 batches per core
P = 128
TSUP = 512                    # timesteps per super tile
JSUB = TSUP // P              # 4 basic 128-t subtiles per super
SUP_PER_GROUP = 4             # supers per softmax group
T_GROUP = TSUP * SUP_PER_GROUP  # 2048
EC = E // P                   # 4 e-chunks
DC = D // P                   # 2 d-chunks
OUTW = EC                     # num cols only (den ships as wb row 0)
GP_CHUNKS = 0                 # numerator e-chunks offloaded to GPSIMD
                              # (0: InstTensorScalarPtr is not codegen-legal
                              # on the Pool/GPSIMD queue - verified)

LAST_RESULT = None            # BassKernelResults of the most recent run


def build_bass(t_loc=T, b_loc=B_LOC, vpool_bufs=4, repeat=1, loop_n=1,
               gp_chunks=GP_CHUNKS):
    """Build the Bass module (same SPMD program for every core)."""
    import concourse.bacc as bacc
    import concourse.tile as tile
    from concourse import mybir

    f32 = mybir.dt.float32
    dtm = mybir.dt.bfloat16

    n_groups = t_loc // T_GROUP
    assert t_loc % T_GROUP == 0

    nc = bacc.Bacc("TRN2", target_bir_lowering=False, debug=False,
                   num_devices=N_CORES)
    valsT = nc.dram_tensor("valuesT", [b_loc, E, t_loc], dtm,
                           kind="ExternalInput").ap()
    w2ed_d = nc.dram_tensor("w2ed", [E, D], dtm, kind="ExternalInput").ap()
    cb_d = nc.dram_tensor("cb", [D, b_loc], f32, kind="ExternalInput").ap()
    vrep_d = nc.dram_tensor("vrep", [D, P], dtm, kind="ExternalInput").ap()
    outp = nc.dram_tensor("out_parts", [b_loc, n_groups, P, OUTW], f32,
                          kind="ExternalOutput").ap()
    wout = nc.dram_tensor("wout", [b_loc, n_groups, T_GROUP], dtm,
                          kind="ExternalOutput").ap()

    with tile.TileContext(nc) as tc:
        _emit(tc, valsT, w2ed_d, cb_d, vrep_d, outp, wout, b_loc,
              n_groups, vpool_bufs, repeat, loop_n, gp_chunks, dtm)
    nc.compile()
    return nc


def _emit(tc, valsT, w2ed_d, cb_d, vrep_d, outp, wout, b_loc,
          n_groups, vpool_bufs, repeat, loop_n, gp_chunks, dtm):
    from contextlib import ExitStack

    from concourse import mybir

    f32 = mybir.dt.float32
    Tanh = mybir.ActivationFunctionType.Tanh
    Exp = mybir.ActivationFunctionType.Exp
    Mult = mybir.AluOpType.mult

    nc = tc.nc

    with ExitStack() as ctx:
        consts = ctx.enter_context(tc.tile_pool(name="consts", bufs=1))
        vtpool = ctx.enter_context(
            tc.tile_pool(name="vtpool", bufs=vpool_bufs))
        thpool = ctx.enter_context(tc.tile_pool(name="thpool", bufs=18))
        bpool = ctx.enter_context(tc.tile_pool(name="bpool", bufs=2))
        scrpool = ctx.enter_context(tc.tile_pool(name="scrpool", bufs=2))
        opool = ctx.enter_context(tc.tile_pool(name="opool", bufs=4))
        ps_vp = ctx.enter_context(
            tc.tile_pool(name="ps_vp", bufs=4, space="PSUM"))
        ps_sm = ctx.enter_context(
            tc.tile_pool(name="ps_sm", bufs=4, space="PSUM"))

        # --- constants ---
        w2_sb = consts.tile([P, EC, D], dtm)
        nc.sync.dma_start(w2_sb, w2ed_d.rearrange("(c p) d -> p c d", p=P))
        cb_sb = consts.tile([P, DC, b_loc], f32)
        nc.sync.dma_start(cb_sb, cb_d.rearrange("(c p) b -> p c b", p=P))
        vrep_sb = consts.tile([P, DC, P], dtm)
        nc.sync.dma_start(vrep_sb, vrep_d.rearrange("(c p) m -> p c m", p=P))

        def emit_weights(p):
            """Prev group's exp: replicated score psum -> weight tile wb
            [128, s, 512] bf16, already in broadcast layout."""
            psss, b, g, rep = p
            wb = bpool.tile([P, SUP_PER_GROUP, TSUP], dtm, tag="wb",
                            name=f"wb_{rep}_{b}_{g}")

            def half(srange):
                for s in srange:
                    nc.scalar.activation(wb[:, s, :], psss[s], Exp)
            return wb, half

        def emit_numerator(p, wb):
            """Prev group's numerator: fused mult+reduce over t, split
            DVE/GPSIMD by e-chunk."""
            vt, b, g, rep = p["vt"], p["b"], p["g"], p["rep"]
            osb = opool.tile([P, OUTW], f32, tag="osb",
                             name=f"osb_{rep}_{b}_{g}")
            scr_v = scrpool.tile([P, T_GROUP], dtm, tag="scrv",
                                 name=f"scrv_{rep}_{b}_{g}")
            wb_flat = wb.rearrange("p s t -> p (s t)")
            for c in range(EC - gp_chunks):
                nc.vector.scalar_tensor_tensor(
                    out=scr_v, in0=vt[:, c, :], scalar=1.0,
                    in1=wb_flat, op0=Mult, op1=Mult,
                    accum_out=osb[:, c:c + 1])
            if gp_chunks:
                scr_g = scrpool.tile([P, T_GROUP], dtm, tag="scrg",
                                     name=f"scrg_{rep}_{b}_{g}")
                for c in range(EC - gp_chunks, EC):
                    nc.gpsimd.scalar_tensor_tensor(
                        out=scr_g, in0=vt[:, c, :], scalar=1.0,
                        in1=wb_flat, op0=Mult, op1=Mult,
                        accum_out=osb[:, c:c + 1])
            nc.sync.dma_start(outp[b, g], osb)
            # ship weight row 0 for the host-side denominator
            nc.sync.dma_start(wout[b, g], wb[0:1].rearrange("p s t -> p (s t)"))

        def emit_scores(p):
            """Prev group's score MMs: deferred a full group so the PE
            never waits on that group's tanh latency. lhsT = replicated v
            [128, 128] -> the score row lands on all 128 psum partitions."""
            for dc in range(DC):
                for s in range(SUP_PER_GROUP):
                    nc.tensor.matmul(p["psss"][s], lhsT=vrep_sb[:, dc, :],
                                     rhs=p["ths"][s][dc],
                                     start=(dc == 0), stop=(dc == DC - 1))

        def body(rep):
          prev = None
          for b in range(b_loc):
            for g in range(n_groups):
                t0g = g * T_GROUP
                vt = vtpool.tile([P, EC, T_GROUP], dtm, tag="vt",
                                 name=f"vt_{rep}_{b}_{g}")
                for c in range(EC):
                    nc.sync.dma_start(
                        vt[:, c, :],
                        valsT[b, c * P:(c + 1) * P,
                              t0g:t0g + T_GROUP])

                # scores psum: one full replicated bank per super
                psss = [ps_sm.tile([P, TSUP], f32, tag="scrow",
                                   name=f"pss{s}_{rep}_{b}_{g}")
                        for s in range(SUP_PER_GROUP)]

                # ---- v_proj, dc-outer so each W2 chunk is loaded into the
                # PE once per pass and reused for 4 super-matmuls ----------
                ths = [[None] * DC for _ in range(SUP_PER_GROUP)]
                psvs = [None] * SUP_PER_GROUP
                # Prev group's deferred score MMs first: they fill the
                # PE while this group's first psv bank waits on the prev
                # dc1 tanh (their own tanh inputs are a full group old).
                if prev is not None:
                    emit_scores(prev)
                for dc in range(DC):
                    for c in range(EC):
                        for s in range(SUP_PER_GROUP):
                            if c == 0:
                                psvs[s] = ps_vp.tile(
                                    [P, TSUP], f32, tag="psv",
                                    name=f"psv_{rep}_{b}_{g}_{s}_{dc}")
                            nc.tensor.matmul(
                                psvs[s],
                                lhsT=w2_sb[:, c, dc * P:(dc + 1) * P],
                                rhs=vt[:, c, s * TSUP:(s + 1) * TSUP],
                                start=(c == 0), stop=(c == EC - 1))
                            if c == EC - 1:
                                th = thpool.tile(
                                    [P, TSUP], dtm, tag="th",
                                    name=f"th_{rep}_{b}_{g}_{s}_{dc}")
                                nc.scalar.activation(
                                    th, psvs[s], Tanh,
                                    bias=cb_sb[:, dc, b:b + 1])
                                ths[s][dc] = th
                    # Prev group's softmax tail: half the exps sit between
                    # the two passes (so the numerator chain starts), half
                    # after dc1's tanhs (so those tanhs are not queued
                    # behind 4 exps - the next group's first psv bank waits
                    # on them).
                    if prev is not None:
                        if dc == 0:
                            wb_p, half_p = emit_weights(prev["soft"])
                            half_p(range(0, 2))
                        else:
                            half_p(range(2, SUP_PER_GROUP))
                            emit_numerator(prev, wb_p)
                            prev = None

                prev = {"vt": vt, "b": b, "g": g, "rep": rep, "ths": ths,
                        "psss": psss, "soft": (psss, b, g, rep)}
          emit_scores(prev)
          wb_p, half_p = emit_weights(prev["soft"])
          half_p(range(SUP_PER_GROUP))
          emit_numerator(prev, wb_p)

        if loop_n > 1:
            with tc.For_i(0, loop_n, 1):
                for rep in range(repeat):
                    body(rep)
        else:
            for rep in range(repeat):
                body(rep)


def host_prepare(values, query, v, W1_w, W1_b, W2_w, W2_b, b_loc=B_LOC,
                 n_cores=N_CORES):
    """Precompute tiny host-side tensors and build per-core input maps."""
    import ml_dtypes

    npm = ml_dtypes.bfloat16

    c = (query.astype(np.float32) @ W1_w.T.astype(np.float32)
         + W1_b + W2_b).astype(np.float32)          # [B, D]
    values_m = np.asarray(values).astype(npm)
    values_t = np.ascontiguousarray(values_m.transpose(0, 2, 1))  # [B, E, T]
    w2ed = np.ascontiguousarray(np.asarray(W2_w).T.astype(npm))  # [E, D]
    vrep = np.ascontiguousarray(
        np.repeat(np.asarray(v).reshape(D, 1), P, axis=1).astype(npm))
    in_maps = []
    for k in range(n_cores):
        bsl = slice(k * b_loc, (k + 1) * b_loc)
        in_maps.append({
            "valuesT": np.ascontiguousarray(values_t[bsl]),
            "w2ed": w2ed,
            "cb": np.ascontiguousarray(c[bsl].T),    # [D, b_loc]
            "vrep": vrep,
        })
    return in_maps


def host_combine(results, b_loc=B_LOC, n_cores=N_CORES):
    """Combine per-(batch, group) partial numerators/denominators.

    out_parts[b, g, p, c] is sum_t w[t] * values[t, c*128+p]; wout[b, g, :]
    holds the bf16 softmax weights of the group (denominator = their sum).
    No max-subtraction: partials are exact exp sums (safe in f32 range).
    """
    out = np.zeros((n_cores * b_loc, E), np.float32)
    for k in range(n_cores):
        num = np.asarray(results[k]["out_parts"]).astype(np.float64)
        den = np.asarray(results[k]["wout"]).astype(np.float64).sum(
            axis=(1, 2))                             # [b]
        numf = num.transpose(0, 1, 3, 2).reshape(b_loc, -1, E)  # e=c*128+p
        o = numf.sum(1) / den[:, None]
        out[k * b_loc:(k + 1) * b_loc] = o.astype(np.float32)
    return out


_NC_CACHE = None


def kernel(values, query, v, W1_w, W1_b, W2_w, W2_b):
    global _NC_CACHE, LAST_RESULT
    from concourse.bass_utils import run_bass_kernel_spmd

    in_maps = host_prepare(values, query, v, W1_w, W1_b, W2_w, W2_b)
    if _NC_CACHE is None:
        _NC_CACHE = build_bass()
    trace = bool(int(os.environ.get("KERNEL_TRACE", "0")))
    LAST_RESULT = run_bass_kernel_spmd(
        _NC_CACHE, in_maps, list(range(N_CORES)), trace=trace)
    return host_combine(LAST_RESULT.results)


if __name__ == "__main__":
    rng = np.random.default_rng(0)
    inputs = {
        "values": rng.standard_normal((B, T, E), dtype=np.float32),
        "query": rng.standard_normal((B, D), dtype=np.float32),
        "v": rng.random(D, dtype=np.float32),
        "W1_w": rng.standard_normal((D, D), dtype=np.float32) * 0.06,
        "W1_b": rng.standard_normal(D, dtype=np.float32) * 0.06,
        "W2_w": rng.standard_normal((D, E), dtype=np.float32) * 0.04,
        "W2_b": rng.standard_normal(D, dtype=np.float32) * 0.04,
    }
    t0 = time.time()
    out = kernel(**inputs)
    print("kernel done in", time.time() - t0, "s", out.shape, out.dtype)


# revision 7
# speedup vs baseline: 1.0592x; 1.0592x over previous
"""Trainium2 Bass kernel for additive attention (nn_AdditiveAttention).

Reference computation (per batch b):
    q_proj = query @ W1_w.T + W1_b                      # [D]
    v_proj = values @ W2_w.T + W2_b                     # [T, D]
    scores = tanh(q_proj + v_proj) @ v                  # [T]
    weights = softmax(scores)                           # [T]
    out    = weights @ values                           # [E]

Sharding: data-parallel over batch B=32 across 8 NeuronCores (4 batches/core).

V8 design (evolves V7). `values` is consumed ONLY in transposed [e, t]
layout (host-pretransposed contiguous 32MB bf16 stream per core). Per group
of tg timesteps (ns = tg/512 supers):

  - DMA vt [128e, 4, tg] bf16
  - v_proj: psum PAIR tiles [128d, 2, 512t] (2 banks), dc-outer /
    e-chunk-inner so each W2 chunk is loaded once per pass
  - ACT tanh over [128, 1024] pairs (amortizes the ~190-cycle op init)
  - scores: lhsT is v REPLICATED into a [128, 128] stationary operand, so
    the score matmul writes the score row to ALL 128 psum partitions for
    the same column count. ns*2 MMs -> psum pair tiles.
  - softmax WITHOUT max subtraction (|scores| <~ 25 so exp fits f32):
    ACT exp on the replicated psum pairs -> weight tile wb [128, ns, 512]
    bf16 DIRECTLY in broadcast layout (no gpsimd broadcast, no accum_out).
    Row 0 of wb ships to HBM; the host sums it for the denominator.
  - numerator: 4 fused mult+reduce [128, tg] DVE ops,
      num_col[p, c] = sum_t vt[p, c, t] * wb[p, t]   (e = c*128+p)
  - per (batch, group) outputs: osb [128, EC] f32 num cols + wb row 0.

The LAST batch is split [2048x3, 1024, 512, 512] so the serial drain tail
(last group's scores+exp+numerator, ~12us for a 2048 group) shrinks to
~4us; earlier batches keep uniform 2048 groups for minimal op overhead.

Engine budget per core: PE ~138us (v_proj 109 + scores 28) <- ceiling,
DVE ~141us, ACT ~100us, DMA ~95us.

All matmul operands are bf16 (~4e-3 rel err; harness gate is 2e-2).
"""

import os
import sys
import time

import numpy as np

for _p in ("/opt/trn_rl_repo",):
    if _p not in sys.path and os.path.isdir(_p):
        sys.path.insert(0, _p)

# Problem shapes (hardcoded per contract)
B, T, E, D = 32, 8192, 512, 256
N_CORES = 8
B_LOC = B // N_CORES          # 4 batches per core
P = 128
TSUP = 512                    # timesteps per super tile
EC = E // P                   # 4 e-chunks
DC = D // P                   # 2 d-chunks
OUTW = EC                     # num cols only (den ships as wb row 0)

GROUPS_STD = (2048, 2048, 2048, 2048)
GROUPS_LAST = (2048, 2048, 2048, 1024, 512, 512)


def batch_groups(b, b_loc=B_LOC):
    return GROUPS_LAST if b == b_loc - 1 else GROUPS_STD


N_CHUNKS = sum(len(batch_groups(b)) for b in range(B_LOC))

LAST_RESULT = None            # BassKernelResults of the most recent run


def build_bass(t_loc=T, b_loc=B_LOC, vpool_bufs=4, repeat=1, loop_n=1):
    """Build the Bass module (same SPMD program for every core)."""
    import concourse.bacc as bacc
    import concourse.tile as tile
    from concourse import mybir

    f32 = mybir.dt.float32
    dtm = mybir.dt.bfloat16

    nc = bacc.Bacc("TRN2", target_bir_lowering=False, debug=False,
                   num_devices=N_CORES)
    valsT = nc.dram_tensor("valuesT", [b_loc, E, t_loc], dtm,
                           kind="ExternalInput").ap()
    w2ed_d = nc.dram_tensor("w2ed", [E, D], dtm, kind="ExternalInput").ap()
    cb_d = nc.dram_tensor("cb", [D, b_loc], f32, kind="ExternalInput").ap()
    vrep_d = nc.dram_tensor("vrep", [D, P], dtm, kind="ExternalInput").ap()
    outp = nc.dram_tensor("out_parts", [N_CHUNKS, P, OUTW], f32,
                          kind="ExternalOutput").ap()
    wout = nc.dram_tensor("wout", [b_loc, t_loc], dtm,
                          kind="ExternalOutput").ap()

    with tile.TileContext(nc) as tc:
        _emit(tc, valsT, w2ed_d, cb_d, vrep_d, outp, wout, b_loc,
              vpool_bufs, repeat, loop_n, dtm)
    nc.compile()
    return nc


def _emit(tc, valsT, w2ed_d, cb_d, vrep_d, outp, wout, b_loc,
          vpool_bufs, repeat, loop_n, dtm):
    from contextlib import ExitStack

    from concourse import mybir

    f32 = mybir.dt.float32
    Tanh = mybir.ActivationFunctionType.Tanh
    Exp = mybir.ActivationFunctionType.Exp
    Mult = mybir.AluOpType.mult

    nc = tc.nc

    with ExitStack() as ctx:
        consts = ctx.enter_context(tc.tile_pool(name="consts", bufs=1))
        vtpool = ctx.enter_context(
            tc.tile_pool(name="vtpool", bufs=vpool_bufs))
        vtpool_s = ctx.enter_context(tc.tile_pool(name="vtpool_s", bufs=2))
        thpool = ctx.enter_context(tc.tile_pool(name="thpool", bufs=9))
        bpool = ctx.enter_context(tc.tile_pool(name="bpool", bufs=2))
        scrpool = ctx.enter_context(tc.tile_pool(name="scrpool", bufs=2))
        opool = ctx.enter_context(tc.tile_pool(name="opool", bufs=4))
        ps_vp = ctx.enter_context(
            tc.tile_pool(name="ps_vp", bufs=2, space="PSUM"))
        ps_sm = ctx.enter_context(
            tc.tile_pool(name="ps_sm", bufs=2, space="PSUM"))

        # --- constants ---
        w2_sb = consts.tile([P, EC, D], dtm)
        nc.sync.dma_start(w2_sb, w2ed_d.rearrange("(c p) d -> p c d", p=P))
        cb_sb = consts.tile([P, DC, b_loc], f32)
        nc.sync.dma_start(cb_sb, cb_d.rearrange("(c p) b -> p c b", p=P))
        vrep_sb = consts.tile([P, DC, P], dtm)
        nc.sync.dma_start(vrep_sb, vrep_d.rearrange("(c p) m -> p c m", p=P))

        def emit_weights(p, pairs):
            """Prev group's exp: replicated score psum pair -> weight tile
            wb slice, bf16, already in broadcast layout."""
            for pair in pairs:
                k = min(2, p["ns"] - 2 * pair)
                nc.scalar.activation(
                    p["wb"][:, 2 * pair:2 * pair + k, :],
                    p["psss"][pair][:, 0:k, :], Exp)

        def emit_numerator(p):
            """Prev group's numerator: 4 fused mult+reduce over t on DVE."""
            vt, b, chunk, rep, tg = (p["vt"], p["b"], p["chunk"], p["rep"],
                                     p["tg"])
            osb = opool.tile([P, OUTW], f32, tag="osb",
                             name=f"osb_{rep}_{chunk}")
            scr_v = scrpool.tile([P, tg], dtm, tag=f"scrv{tg}",
                                 name=f"scrv_{rep}_{chunk}")
            wb_flat = p["wb"].rearrange("p s t -> p (s t)")
            for c in range(EC):
                nc.vector.scalar_tensor_tensor(
                    out=scr_v, in0=vt[:, c, :], scalar=1.0,
                    in1=wb_flat, op0=Mult, op1=Mult,
                    accum_out=osb[:, c:c + 1])
            nc.sync.dma_start(outp[chunk], osb)
            # ship weight row 0 for the host-side denominator
            nc.sync.dma_start(wout[b, p["t0"]:p["t0"] + tg],
                              p["wb"][0:1].rearrange("p s t -> p (s t)"))

        def emit_scores(p):
            """Prev group's score MMs: deferred a full group so the PE
            never waits on that group's tanh latency. lhsT = replicated v
            [128, 128] -> the score row lands on all 128 psum partitions."""
            for dc in range(DC):
                for s in range(p["ns"]):
                    nc.tensor.matmul(
                        p["psss"][s // 2][:, s % 2, :],
                        lhsT=vrep_sb[:, dc, :],
                        rhs=p["ths"][s // 2][dc][:, s % 2, :],
                        start=(dc == 0), stop=(dc == DC - 1))

        def body(rep):
          prev = None
          chunk = 0
          for b in range(b_loc):
            t0 = 0
            for tg in batch_groups(b, b_loc):
                ns = tg // TSUP
                npair = (ns + 1) // 2
                pool = vtpool if tg == 2048 else vtpool_s
                vt = pool.tile([P, EC, tg], dtm, tag=f"vt{tg}",
                               name=f"vt_{rep}_{chunk}")
                for c in range(EC):
                    nc.sync.dma_start(
                        vt[:, c, :],
                        valsT[b, c * P:(c + 1) * P, t0:t0 + tg])

                # scores psum: replicated pair tiles (2 banks each)
                psss = [ps_sm.tile([P, 2, TSUP], f32, tag="scrow",
                                   name=f"pss{pr}_{rep}_{chunk}")
                        for pr in range(npair)]
                wb = bpool.tile([P, ns, TSUP], dtm, tag=f"wb{ns}",
                                name=f"wb_{rep}_{chunk}")

                # ---- v_proj, dc-outer so each W2 chunk is loaded into the
                # PE once per pass and reused for ns super-matmuls ---------
                ths = [[None] * DC for _ in range(npair)]
                psvs = [None] * npair
                # Prev group's deferred score MMs first: they fill the
                # PE while this group's first psv bank waits on the prev
                # dc1 tanh (their own tanh inputs are a full group old).
                if prev is not None:
                    emit_scores(prev)
                for dc in range(DC):
                    for c in range(EC):
                        for s in range(ns):
                            if c == 0 and s % 2 == 0:
                                psvs[s // 2] = ps_vp.tile(
                                    [P, 2, TSUP], f32, tag="psv",
                                    name=f"psv_{rep}_{chunk}_{s}_{dc}")
                            nc.tensor.matmul(
                                psvs[s // 2][:, s % 2, :],
                                lhsT=w2_sb[:, c, dc * P:(dc + 1) * P],
                                rhs=vt[:, c, s * TSUP:(s + 1) * TSUP],
                                start=(c == 0), stop=(c == EC - 1))
                            if c == EC - 1 and (s % 2 == 1 or s == ns - 1):
                                pr = s // 2
                                k = s % 2 + 1
                                th = thpool.tile(
                                    [P, 2, TSUP], dtm, tag="th",
                                    name=f"th_{rep}_{chunk}_{pr}_{dc}")
                                nc.scalar.activation(
                                    th[:, 0:k, :], psvs[pr][:, 0:k, :],
                                    Tanh, bias=cb_sb[:, dc, b:b + 1])
                                ths[pr][dc] = th
                    # Prev group's softmax tail: first exp pair between the
                    # two passes (so the numerator chain starts), the rest
                    # after dc1's tanhs (so those tanhs are not queued
                    # behind the exps - the next group's first psv bank
                    # waits on them).
                    if prev is not None:
                        if dc == 0:
                            emit_weights(prev, range(0, 1))
                        else:
                            emit_weights(prev, range(1, prev["npair"]))
                            emit_numerator(prev)
                            prev = None

                prev = {"vt": vt, "b": b, "chunk": chunk, "rep": rep,
                        "ths": ths, "psss": psss, "wb": wb, "ns": ns,
                        "npair": npair, "tg": tg, "t0": t0}
                chunk += 1
                t0 += tg
          emit_scores(prev)
          emit_weights(prev, range(prev["npair"]))
          emit_numerator(prev)

        if loop_n > 1:
            with tc.For_i(0, loop_n, 1):
                for rep in range(repeat):
                    body(rep)
        else:
            for rep in range(repeat):
                body(rep)


def host_prepare(values, query, v, W1_w, W1_b, W2_w, W2_b, b_loc=B_LOC,
                 n_cores=N_CORES):
    """Precompute tiny host-side tensors and build per-core input maps."""
    import ml_dtypes

    npm = ml_dtypes.bfloat16

    c = (query.astype(np.float32) @ W1_w.T.astype(np.float32)
         + W1_b + W2_b).astype(np.float32)          # [B, D]
    values_m = np.asarray(values).astype(npm)
    values_t = np.ascontiguousarray(values_m.transpose(0, 2, 1))  # [B, E, T]
    w2ed = np.ascontiguousarray(np.asarray(W2_w).T.astype(npm))  # [E, D]
    vrep = np.ascontiguousarray(
        np.repeat(np.asarray(v).reshape(D, 1), P, axis=1).astype(npm))
    in_maps = []
    for k in range(n_cores):
        bsl = slice(k * b_loc, (k + 1) * b_loc)
        in_maps.append({
            "valuesT": np.ascontiguousarray(values_t[bsl]),
            "w2ed": w2ed,
            "cb": np.ascontiguousarray(c[bsl].T),    # [D, b_loc]
            "vrep": vrep,
        })
    return in_maps


def host_combine(results, b_loc=B_LOC, n_cores=N_CORES):
    """Combine per-(batch, group) partial numerators/denominators.

    out_parts[chunk, p, c] is sum_t w[t] * values[t, c*128+p] over that
    group's t-range; wout[b, :] holds the bf16 softmax weights
    (denominator = their sum). No max-subtraction: partials are exact exp
    sums (safe in f32 range).
    """
    out = np.zeros((n_cores * b_loc, E), np.float32)
    for k in range(n_cores):
        num = np.asarray(results[k]["out_parts"]).astype(np.float64)
        den = np.asarray(results[k]["wout"]).astype(np.float64).sum(1)  # [b]
        chunk = 0
        for b in range(b_loc):
            nb = num[chunk:chunk + len(batch_groups(b, b_loc))]
            chunk += len(batch_groups(b, b_loc))
            numf = nb.transpose(0, 2, 1).reshape(-1, E)     # e = c*128+p
            out[k * b_loc + b] = (numf.sum(0) / den[b]).astype(np.float32)
    return out


_NC_CACHE = None


def kernel(values, query, v, W1_w, W1_b, W2_w, W2_b):
    global _NC_CACHE, LAST_RESULT
    from concourse.bass_utils import run_bass_kernel_spmd

    in_maps = host_prepare(values, query, v, W1_w, W1_b, W2_w, W2_b)
    if _NC_CACHE is None:
        _NC_CACHE = build_bass()
    trace = bool(int(os.environ.get("KERNEL_TRACE", "0")))
    LAST_RESULT = run_bass_kernel_spmd(
        _NC_CACHE, in_maps, list(range(N_CORES)), trace=trace)
    return host_combine(LAST_RESULT.results)


if __name__ == "__main__":
    rng = np.random.default_rng(0)
    inputs = {
        "values": rng.standard_normal((B, T, E), dtype=np.float32),
        "query": rng.standard_normal((B, D), dtype=np.float32),
        "v": rng.random(D, dtype=np.float32),
        "W1_w": rng.standard_normal((D, D), dtype=np.float32) * 0.06,
        "W1_b": rng.standard_normal(D, dtype=np.float32) * 0.06,
        "W2_w": rng.standard_normal((D, E), dtype=np.float32) * 0.04,
        "W2_b": rng.standard_normal(D, dtype=np.float32) * 0.04,
    }
    t0 = time.time()
    out = kernel(**inputs)
    print("kernel done in", time.time() - t0, "s", out.shape, out.dtype)


# revision 9
# speedup vs baseline: 1.0999x; 1.0385x over previous
"""Trainium2 Bass kernel for additive attention (nn_AdditiveAttention).

Reference computation (per batch b):
    q_proj = query @ W1_w.T + W1_b                      # [D]
    v_proj = values @ W2_w.T + W2_b                     # [T, D]
    scores = tanh(q_proj + v_proj) @ v                  # [T]
    weights = softmax(scores)                           # [T]
    out    = weights @ values                           # [E]

Sharding: data-parallel over batch B=32 across 8 NeuronCores (4 batches/core).

V7 design (evolves V6). `values` is consumed ONLY in transposed [e, t]
layout (host-pretransposed contiguous 32MB bf16 stream per core). Per group
of 2048 timesteps:

  - DMA vt [128e, 4, 2048t] bf16 (one 2MB load)
  - v_proj: 4 psum tiles [128d(chunk dc), 512t], dc-outer / e-chunk-inner so
    each W2 chunk is loaded once per pass and reused for 4 super matmuls
  - ACT tanh w/ per-partition bias -> th bf16
  - scores: lhsT is v REPLICATED into a [128, 128] stationary operand, so
    the score matmul writes the score row to ALL 128 psum partitions for the
    same column count (cost unchanged). 8 MMs per group -> 4 full
    [128, 512] psum banks.
  - softmax WITHOUT max subtraction (|scores| <~ 25 so exp fits f32):
    ACT exp on the replicated psum -> weight tile wb [128, 4, 512] bf16
    DIRECTLY in broadcast layout (the V6 gpsimd partition_broadcast and the
    exp accum_out are both gone). Row 0 of wb is DMA'd out per group; the
    host sums it for the denominator.
  - numerator: 4 fused mult+reduce [128, 2048] ops,
      num_col[p, c] = sum_t vt[p, c, t] * wb[p, t]   (e = c*128+p)
    split 3 on DVE + 1 on GPSIMD (engine budget: DVE ~105us, Pool ~47us).
  - per (batch, group) outputs: osb [128, EC] f32 num cols + wb row 0.

Engine budget per core: PE ~138us (v_proj 109 + scores 28) <- ceiling,
DVE ~105us, ACT ~101us (tanh 65 + exp 36), DMA ~95us, Pool ~47us.

All matmul operands are bf16 (~4e-3 rel err; harness gate is 2e-2).
"""

import os
import sys
import time

import numpy as np

for _p in ("/opt/trn_rl_repo",):
    if _p not in sys.path and os.path.isdir(_p):
        sys.path.insert(0, _p)

# Problem shapes (hardcoded per contract)
B, T, E, D = 32, 8192, 512, 256
N_CORES = 8
B_LOC = B // N_CORES          # 4 batches per core
P = 128
TSUP = 512                    # timesteps per super tile
EC = E // P                   # 4 e-chunks
DC = D // P                   # 2 d-chunks
OUTW = EC                     # num cols only (den ships as wb row 0)
GROUPS_STD = (2048, 2048, 2048, 2048)
GROUPS_LAST = (2048, 2048, 2048, 1024, 512, 512)


def batch_groups(b, b_loc=B_LOC):
    """The last batch tapers off so the pipeline drain tail is short."""
    return GROUPS_LAST if b == b_loc - 1 else GROUPS_STD


N_CHUNKS = sum(len(batch_groups(b)) for b in range(B_LOC))

LAST_RESULT = None            # BassKernelResults of the most recent run


def build_bass(t_loc=T, b_loc=B_LOC, vpool_bufs=4, repeat=1, loop_n=1):
    """Build the Bass module (same SPMD program for every core)."""
    import concourse.bacc as bacc
    import concourse.tile as tile
    from concourse import mybir

    f32 = mybir.dt.float32
    dtm = mybir.dt.bfloat16

    nc = bacc.Bacc("TRN2", target_bir_lowering=False, debug=False,
                   num_devices=N_CORES)
    valsT = nc.dram_tensor("valuesT", [b_loc, E, t_loc], dtm,
                           kind="ExternalInput").ap()
    w2ed_d = nc.dram_tensor("w2ed", [E, D], dtm, kind="ExternalInput").ap()
    cb_d = nc.dram_tensor("cb", [D, b_loc], f32, kind="ExternalInput").ap()
    vrep_d = nc.dram_tensor("vrep", [D, P], dtm, kind="ExternalInput").ap()
    outp = nc.dram_tensor("out_parts", [N_CHUNKS, P, OUTW], f32,
                          kind="ExternalOutput").ap()
    wout = nc.dram_tensor("wout", [b_loc, t_loc], dtm,
                          kind="ExternalOutput").ap()

    with tile.TileContext(nc) as tc:
        _emit(tc, valsT, w2ed_d, cb_d, vrep_d, outp, wout, b_loc,
              vpool_bufs, repeat, loop_n, dtm)
    nc.compile()
    return nc


def _emit(tc, valsT, w2ed_d, cb_d, vrep_d, outp, wout, b_loc,
          vpool_bufs, repeat, loop_n, dtm):
    from contextlib import ExitStack

    from concourse import mybir

    f32 = mybir.dt.float32
    Tanh = mybir.ActivationFunctionType.Tanh
    Exp = mybir.ActivationFunctionType.Exp
    Mult = mybir.AluOpType.mult

    nc = tc.nc

    with ExitStack() as ctx:
        consts = ctx.enter_context(tc.tile_pool(name="consts", bufs=1))
        vtpool = ctx.enter_context(
            tc.tile_pool(name="vtpool", bufs=vpool_bufs))
        vtpool_s = ctx.enter_context(tc.tile_pool(name="vtpool_s", bufs=2))
        thpool = ctx.enter_context(tc.tile_pool(name="thpool", bufs=18))
        bpool = ctx.enter_context(tc.tile_pool(name="bpool", bufs=2))
        scrpool = ctx.enter_context(tc.tile_pool(name="scrpool", bufs=2))
        opool = ctx.enter_context(tc.tile_pool(name="opool", bufs=4))
        ps_vp = ctx.enter_context(
            tc.tile_pool(name="ps_vp", bufs=4, space="PSUM"))
        ps_sm = ctx.enter_context(
            tc.tile_pool(name="ps_sm", bufs=4, space="PSUM"))

        # --- constants ---
        w2_sb = consts.tile([P, EC, D], dtm)
        nc.sync.dma_start(w2_sb, w2ed_d.rearrange("(c p) d -> p c d", p=P))
        cb_sb = consts.tile([P, DC, b_loc], f32)
        nc.sync.dma_start(cb_sb, cb_d.rearrange("(c p) b -> p c b", p=P))
        vrep_sb = consts.tile([P, DC, P], dtm)
        nc.sync.dma_start(vrep_sb, vrep_d.rearrange("(c p) m -> p c m", p=P))

        def emit_weights(p, srange):
            """Prev group's exp: replicated score psum -> weight tile wb
            slice, bf16, already in broadcast layout."""
            for s in srange:
                nc.scalar.activation(p["wb"][:, s, :], p["psss"][s], Exp)

        def emit_numerator(p):
            """Prev group's numerator: fused mult+reduce over t on DVE."""
            vt, tg, chunk, rep = p["vt"], p["tg"], p["chunk"], p["rep"]
            osb = opool.tile([P, OUTW], f32, tag="osb",
                             name=f"osb_{rep}_{chunk}")
            scr_v = scrpool.tile([P, tg], dtm, tag=f"scrv{tg}",
                                 name=f"scrv_{rep}_{chunk}")
            wb_flat = p["wb"].rearrange("p s t -> p (s t)")
            for c in range(EC):
                nc.vector.scalar_tensor_tensor(
                    out=scr_v, in0=vt[:, c, :], scalar=1.0,
                    in1=wb_flat, op0=Mult, op1=Mult,
                    accum_out=osb[:, c:c + 1])
            nc.sync.dma_start(outp[chunk], osb)
            # ship weight row 0 for the host-side denominator
            nc.sync.dma_start(wout[p["b"], p["t0"]:p["t0"] + tg],
                              p["wb"][0:1].rearrange("p s t -> p (s t)"))

        def emit_scores(p):
            """Prev group's score MMs: deferred a full group so the PE
            never waits on that group's tanh latency. lhsT = replicated v
            [128, 128] -> the score row lands on all 128 psum partitions."""
            for dc in range(DC):
                for s in range(p["ns"]):
                    nc.tensor.matmul(p["psss"][s], lhsT=vrep_sb[:, dc, :],
                                     rhs=p["ths"][s][dc],
                                     start=(dc == 0), stop=(dc == DC - 1))

        def body(rep):
          prev = None
          chunk = 0
          for b in range(b_loc):
            t0 = 0
            for tg in batch_groups(b, b_loc):
                ns = tg // TSUP
                pool = vtpool if tg == 2048 else vtpool_s
                vt = pool.tile([P, EC, tg], dtm, tag=f"vt{tg}",
                               name=f"vt_{rep}_{chunk}")
                for c in range(EC):
                    nc.sync.dma_start(
                        vt[:, c, :],
                        valsT[b, c * P:(c + 1) * P, t0:t0 + tg])

                # scores psum: one full replicated bank per super
                psss = [ps_sm.tile([P, TSUP], f32, tag="scrow",
                                   name=f"pss{s}_{rep}_{chunk}")
                        for s in range(ns)]
                wb = bpool.tile([P, ns, TSUP], dtm, tag=f"wb{ns}",
                                name=f"wb_{rep}_{chunk}")

                # ---- v_proj, dc-outer so each W2 chunk is loaded into the
                # PE once per pass and reused for ns super-matmuls ---------
                ths = [[None] * DC for _ in range(ns)]
                psvs = [None] * ns
                # Prev group's deferred score MMs first: they fill the
                # PE while this group's first psv bank waits on the prev
                # dc1 tanh (their own tanh inputs are a full group old).
                if prev is not None:
                    emit_scores(prev)
                for dc in range(DC):
                    for c in range(EC):
                        for s in range(ns):
                            if c == 0:
                                psvs[s] = ps_vp.tile(
                                    [P, TSUP], f32, tag="psv",
                                    name=f"psv_{rep}_{chunk}_{s}_{dc}")
                            nc.tensor.matmul(
                                psvs[s],
                                lhsT=w2_sb[:, c, dc * P:(dc + 1) * P],
                                rhs=vt[:, c, s * TSUP:(s + 1) * TSUP],
                                start=(c == 0), stop=(c == EC - 1))
                            if c == EC - 1:
                                th = thpool.tile(
                                    [P, TSUP], dtm, tag="th",
                                    name=f"th_{rep}_{chunk}_{s}_{dc}")
                                nc.scalar.activation(
                                    th, psvs[s], Tanh,
                                    bias=cb_sb[:, dc, b:b + 1])
                                ths[s][dc] = th
                    # Prev group's softmax tail: half the exps sit between
                    # the two passes (so the numerator chain starts), half
                    # after dc1's tanhs (so those tanhs are not queued
                    # behind the exps - the next group's first psv bank
                    # waits on them).
                    if prev is not None:
                        if dc == 0:
                            emit_weights(prev, range(0, (prev["ns"] + 1) // 2))
                        else:
                            emit_weights(prev,
                                         range((prev["ns"] + 1) // 2,
                                               prev["ns"]))
                            emit_numerator(prev)
                            prev = None

                prev = {"vt": vt, "b": b, "chunk": chunk, "rep": rep,
                        "ths": ths, "psss": psss, "wb": wb, "ns": ns,
                        "tg": tg, "t0": t0}
                chunk += 1
                t0 += tg
          emit_scores(prev)
          emit_weights(prev, range(prev["ns"]))
          emit_numerator(prev)

        if loop_n > 1:
            with tc.For_i(0, loop_n, 1):
                for rep in range(repeat):
                    body(rep)
        else:
            for rep in range(repeat):
                body(rep)


def host_prepare(values, query, v, W1_w, W1_b, W2_w, W2_b, b_loc=B_LOC,
                 n_cores=N_CORES):
    """Precompute tiny host-side tensors and build per-core input maps."""
    import ml_dtypes

    npm = ml_dtypes.bfloat16

    c = (query.astype(np.float32) @ W1_w.T.astype(np.float32)
         + W1_b + W2_b).astype(np.float32)          # [B, D]
    values_m = np.asarray(values).astype(npm)
    values_t = np.ascontiguousarray(values_m.transpose(0, 2, 1))  # [B, E, T]
    w2ed = np.ascontiguousarray(np.asarray(W2_w).T.astype(npm))  # [E, D]
    vrep = np.ascontiguousarray(
        np.repeat(np.asarray(v).reshape(D, 1), P, axis=1).astype(npm))
    in_maps = []
    for k in range(n_cores):
        bsl = slice(k * b_loc, (k + 1) * b_loc)
        in_maps.append({
            "valuesT": np.ascontiguousarray(values_t[bsl]),
            "w2ed": w2ed,
            "cb": np.ascontiguousarray(c[bsl].T),    # [D, b_loc]
            "vrep": vrep,
        })
    return in_maps


def host_combine(results, b_loc=B_LOC, n_cores=N_CORES):
    """Combine per-(batch, group) partial numerators/denominators.

    out_parts[b, g, p, c] is sum_t w[t] * values[t, c*128+p]; wout[b, g, :]
    holds the bf16 softmax weights of the group (denominator = their sum).
    No max-subtraction: partials are exact exp sums (safe in f32 range).
    """
    out = np.zeros((n_cores * b_loc, E), np.float32)
    for k in range(n_cores):
        num = np.asarray(results[k]["out_parts"]).astype(np.float64)
        den = np.asarray(results[k]["wout"]).astype(np.float64).sum(1)  # [b]
        chunk = 0
        for b in range(b_loc):
            ng = len(batch_groups(b, b_loc))
            nb = num[chunk:chunk + ng]
            chunk += ng
            numf = nb.transpose(0, 2, 1).reshape(-1, E)     # e = c*128+p
            out[k * b_loc + b] = (numf.sum(0) / den[b]).astype(np.float32)
    return out


_NC_CACHE = None


def kernel(values, query, v, W1_w, W1_b, W2_w, W2_b):
    global _NC_CACHE, LAST_RESULT
    from concourse.bass_utils import run_bass_kernel_spmd

    in_maps = host_prepare(values, query, v, W1_w, W1_b, W2_w, W2_b)
    if _NC_CACHE is None:
        _NC_CACHE = build_bass()
    trace = bool(int(os.environ.get("KERNEL_TRACE", "0")))
    LAST_RESULT = run_bass_kernel_spmd(
        _NC_CACHE, in_maps, list(range(N_CORES)), trace=trace)
    return host_combine(LAST_RESULT.results)


if __name__ == "__main__":
    rng = np.random.default_rng(0)
    inputs = {
        "values": rng.standard_normal((B, T, E), dtype=np.float32),
        "query": rng.standard_normal((B, D), dtype=np.float32),
        "v": rng.random(D, dtype=np.float32),
        "W1_w": rng.standard_normal((D, D), dtype=np.float32) * 0.06,
        "W1_b": rng.standard_normal(D, dtype=np.float32) * 0.06,
        "W2_w": rng.standard_normal((D, E), dtype=np.float32) * 0.04,
        "W2_b": rng.standard_normal(D, dtype=np.float32) * 0.04,
    }
    t0 = time.time()
    out = kernel(**inputs)
    print("kernel done in", time.time() - t0, "s", out.shape, out.dtype)
